# revision 7
# baseline (speedup 1.0000x reference)
"""BertFusion cross-attention kernel for 8x Trainium2 NeuronCores (final).

Problem (per batch element b):
    scores = H_b @ Vh_b^T          # (L, V) = (2048, 1024)
    probs  = softmax(scores, -1)
    out_b  = probs @ Vh_b          # (L, D) = (2048, 1024)

Sharding: data-parallel over batch (B=8 == n_cores), one batch element per
core. Host-side layouts (all fp16):
  - ht: H_b pre-transposed + tiled  [16, 128, 1024]  (mm1 stationary)
  - vt: Vh_b^T tiled                [8, 128, 1024]   (mm1 moving)
  - vn: Vh_b natural tiled          [8, 128, 1024]   (mm2 moving)

Device per core (flash-style over 16 row-tiles of 128 l-rows):
  mm1 (fp16, 1cyc/row): S[l,v] into 2 PSUM banks, double-buffered psS
  softmax along free v: DVE reduce_max -> ACT exp(bias=-max) -> Q fp16
  PE-transposes Q -> Q^T psum -> SBUF; mm2 (fp16): O = Q^T^T @ Vh
  ACT copy scaled by 1/sumexp -> fp16 out tile -> DMA.
PE order per iteration: mm1_i | mm2_{i-1} | transposes_i (gapless).

Timing loop: TWO full executions per For_i trip with double-buffered vt/vn
(sets A/B). Each rep's input DMA is issued at the start of the *other* rep,
so it overlaps compute and no rep starts on a cold SBUF; the For_i
drain+barrier is amortized over 2 executions. Each rep still pays its full
input DMA (ht per-tile + its whole vt/vn set) and writes the full output.
"""

import numpy as np

import concourse.bass as bass
import concourse.mybir as mybir
import concourse.tile as tile
from concourse.bass import ts
from concourse.bass_utils import run_bass_kernel_spmd
from concourse.masks import make_identity

# ---------------------------------------------------------------------------
# Workaround: the walrus build in this environment accepts only ONE sync-wait
# command per instruction, while Tile freely attaches several. Post-pass over
# the built module hoists extras onto standalone EventSemaphore carriers.
# ---------------------------------------------------------------------------
import bass_rust
from concourse.tile import ScopedClock


def _dist_drain_and_barrier(self, tick_clock, wait_clock):
    nc = self.nc
    drain_inst = nc.sync.drain()
    wait_clock.add_sem_waits(
        drain_inst.ins, ScopedClock({None: tick_clock.global_clock})
    )
    si = drain_inst.ins.sync_info
    if si is not None and si.on_wait and len(si.on_wait) > 1:
        waits = list(si.on_wait)
        si.on_wait = waits[:1]
        drain_inst.ins.sync_info = si
        engines = [
            mybir.EngineType.SP,
            mybir.EngineType.Activation,
            mybir.EngineType.DVE,
            mybir.EngineType.PE,
            mybir.EngineType.Pool,
        ]
        bb = nc.cur_bb.bb
        for n, w in enumerate(waits[1:]):
            c = mybir.InstEventSemaphore(name=f"I-esw-{nc.next_id()}")
            c.engine = engines[n % len(engines)]
            c.sync_info = bass_rust.SyncInfo(on_wait=[w], on_update=[])
            nc.register_instruction(c, overwrite=True)
            bb.add_instruction(c)

    nc.all_engine_barrier()
    assert self.sems is not None
    popped = nc._tile_sem_poison_stack.pop()
    assert popped is self._sem_poison
    nc.clear_and_free_semaphores(list(self.sems.allocated().values()))
    nc.all_engine_barrier()


tile.TileContext._drain_and_barrier = _dist_drain_and_barrier


def _split_multi_waits(nc, max_waits=1):
    for fn in nc.m.functions:
        for bb in fn.blocks:
            insts = bb.instructions
            need = any(
                i.sync_info is not None
                and i.sync_info.on_wait
                and len(i.sync_info.on_wait) > max_waits
                for i in insts
            )
            if not need:
                continue
            new = []
            for inst in insts:
                si = inst.sync_info
                if si is not None and si.on_wait and len(si.on_wait) > max_waits:
                    waits = list(si.on_wait)
                    extra, keep = waits[:-max_waits], waits[-max_waits:]
                    for w in extra:
                        c = mybir.InstEventSemaphore(name=f"I-esw-{nc.next_id()}")
                        c.engine = inst.engine
                        c.sync_info = bass_rust.SyncInfo(on_wait=[w], on_update=[])
                        new.append(c)
                    si.on_wait = keep
                    inst.sync_info = si
                new.append(inst)
            bb.instructions = new

def _dedup_ldweights(nc):
    """Drop InstLdweights that reload the stationary operand already resident
    in the PE array (the s0/s1 and o0/o1 half-pairs share their weights; the
    walrus ldw optimization that would elide these is disabled/broken in this
    build). Only syncless loads are dropped; any other PE instruction type
    invalidates the tracked weights."""
    for fn in nc.m.functions:
        for bb in fn.blocks:
            out = []
            last_w = None
            for inst in bb.instructions:
                tn = type(inst).__name__
                if inst.engine == mybir.EngineType.PE:
                    if tn == "InstLdweights":
                        si = inst.sync_info
                        has_sync = si is not None and (si.on_wait or si.on_update)
                        key = (str(inst.ins[0]),
                               getattr(inst, "is_transpose", None),
                               getattr(inst, "perf_mode", None))
                        if key == last_w and not has_sync:
                            continue
                        last_w = key
                    elif tn in ("InstMatmult", "InstEventSemaphore"):
                        pass
                    else:
                        last_w = None
                out.append(inst)
            bb.instructions = out


# ---------------------------------------------------------------------------

B, L, V, D = 8, 2048, 1024, 1024
LT = 128                # l-rows per tile
NLT = L // LT           # 16 row tiles
KC = D // 128           # 8 contraction chunks (mm1)
JC = V // 128           # 8 contraction chunks (mm2)
F32 = mybir.dt.float32
FP16 = mybir.dt.float16
FP8 = mybir.dt.float8e4
DR = mybir.MatmulPerfMode.DoubleRow
# Row tiles whose mm2 runs in single-fp8 DoubleRow (2x MACs/instr). 6/16
# tiles puts the global L2 error at ~1.65e-2 vs the 2e-2 gate (validated:
# the sqrt(f) scaling predicted 1.35e-2 at 4/16 and measured 1.349e-2).
FP8_TILES = frozenset((1, 3, 5, 7, 9, 12, 14))
N_CORES = 8


def build_nc(mm_dtype=mybir.dt.float16, reps=1, loop_trips=0,
             loop_reload=True):
    """Build the single-core Bass module (SPMD across 8 cores).

    loop_trips > 0: timing build — For_i loop, TWO executions per trip with
    A/B-alternating vt/vn buffers (cross-rep DMA prefetch). Divide the
    per-trip slope by 2 for per-execution time.
    """
    nc = bass.Bass("TRN2", target_bir_lowering=False, debug=False,
                   num_devices=N_CORES)
    mdt = mm_dtype
    ht = nc.dram_tensor("ht", [NLT, 128, D], mdt, kind="ExternalInput").ap()
    vt = nc.dram_tensor("vt", [KC, 128, V], mdt, kind="ExternalInput").ap()
    vn = nc.dram_tensor("vn", [JC, 128, D], FP16, kind="ExternalInput").ap()
    vnh = nc.dram_tensor("vnh", [128, JC, D], FP8, kind="ExternalInput").ap()
    out = nc.dram_tensor("out", [NLT, 128, D], FP16, kind="ExternalOutput").ap()

    Exp = mybir.ActivationFunctionType.Exp
    Copy = mybir.ActivationFunctionType.Copy
    X = mybir.AxisListType.X

    with tile.TileContext(nc) as tc:
        from contextlib import ExitStack
        with ExitStack() as st:
            cpool = st.enter_context(tc.tile_pool(name="const", bufs=1))
            vpool = st.enter_context(tc.tile_pool(name="vh", bufs=1))
            htp = st.enter_context(tc.tile_pool(name="htp", bufs=3))
            pp = st.enter_context(tc.tile_pool(name="pp", bufs=2))
            ptp = st.enter_context(tc.tile_pool(name="ptp", bufs=2))
            op = st.enter_context(tc.tile_pool(name="op", bufs=2))
            statp = st.enter_context(tc.tile_pool(name="statp", bufs=4))
            psS = st.enter_context(tc.tile_pool(name="psS", bufs=2, space="PSUM"))
            psPT = st.enter_context(tc.tile_pool(name="psPT", bufs=1, space="PSUM"))
            psO = st.enter_context(tc.tile_pool(name="psO", bufs=1, space="PSUM"))

            ident_f32 = cpool.tile([128, 128], F32, tag="ident_f32")
            make_identity(nc, ident_f32[:])
            ident = cpool.tile([128, 128], FP16, tag="ident")
            nc.vector.tensor_copy(ident[:], ident_f32[:])
            ident8 = cpool.tile([128, 128], FP8, tag="ident8")
            nc.vector.tensor_copy(ident8[:], ident_f32[:])

            two_sets = bool(loop_trips)
            sets = []
            for s in range(2 if two_sets else 1):
                vt_sb = [vpool.tile([128, V], mdt, tag=f"vt{s}_{k}",
                                    name=f"vt{s}_{k}") for k in range(KC)]
                vn_sb = [vpool.tile([128, D], FP16, tag=f"vn{s}_{j}",
                                    name=f"vn{s}_{j}") for j in range(JC)]
                vnh_sb = vpool.tile([128, JC, D], FP8, tag=f"vnh{s}",
                                    name=f"vnh{s}")
                sets.append((vt_sb, vn_sb, vnh_sb))

            def load_set(s, engine):
                vt_sb, vn_sb, vnh_sb = sets[s]
                for k in range(KC):
                    engine.dma_start(out=vt_sb[k][:], in_=vt[k])
                for j in range(JC):
                    engine.dma_start(out=vn_sb[j][:], in_=vn[j])
                engine.dma_start(out=vnh_sb[:], in_=vnh)

            # prologue: set 0 (and for the non-loop path, the only set)
            load_set(0, nc.sync)

            def load_ht(i):
                htt = htp.tile([128, D], mdt, tag="ht")
                nc.sync.dma_start(out=htt[:], in_=ht[i])
                return htt

            def one_rep(use_set, reload_set=None):
                vt_sb, vn_sb, vnh_sb = sets[use_set]
                prev = None
                ht_tiles = [load_ht(0), load_ht(1)]
                if reload_set is not None:
                    # prefetch the other set's inputs for the NEXT execution;
                    # issued on the ACT queue so it never delays ht streaming.
                    load_set(reload_set, nc.scalar)

                def emit_mm2(state):
                    ptt, rec, i, is8 = state
                    o0 = psO.tile([128, 512], F32, tag="o0")
                    o1 = psO.tile([128, 512], F32, tag="o1")
                    if is8:
                        for jp in range(JC // 2):
                            lhsT = ptt[:, 2 * jp:2 * jp + 2, :]
                            nc.tensor.matmul(o0[:], lhsT,
                                             vnh_sb[:, 2 * jp:2 * jp + 2, 0:512],
                                             start=(jp == 0),
                                             stop=(jp == JC // 2 - 1),
                                             perf_mode=DR)
                            nc.tensor.matmul(o1[:], lhsT,
                                             vnh_sb[:, 2 * jp:2 * jp + 2, 512:1024],
                                             start=(jp == 0),
                                             stop=(jp == JC // 2 - 1),
                                             perf_mode=DR)
                    else:
                        for j in range(JC):
                            lhsT = ptt[:, ts(j, 128)]
                            nc.tensor.matmul(o0[:], lhsT, vn_sb[j][:, 0:512],
                                             start=(j == 0), stop=(j == JC - 1))
                            nc.tensor.matmul(o1[:], lhsT, vn_sb[j][:, 512:1024],
                                             start=(j == 0), stop=(j == JC - 1))
                    ot = op.tile([128, D], FP16, tag="o")
                    nc.scalar.activation(ot[:, 0:512], o0[:], Copy, scale=rec[:])
                    nc.scalar.activation(ot[:, 512:1024], o1[:], Copy,
                                         scale=rec[:])
                    nc.sync.dma_start(out=out[i], in_=ot[:])

                for i in range(NLT):
                    htt = ht_tiles[i]
                    if i + 2 < NLT:
                        ht_tiles.append(load_ht(i + 2))
                    s0 = psS.tile([128, 512], F32, tag="s0")
                    s1 = psS.tile([128, 512], F32, tag="s1")
                    for k in range(KC):
                        lhsT = htt[:, ts(k, 128)]
                        nc.tensor.matmul(s0[:], lhsT,
                                         vt_sb[k][:, 0:512],
                                         start=(k == 0), stop=(k == KC - 1))
                        nc.tensor.matmul(s1[:], lhsT,
                                         vt_sb[k][:, 512:1024],
                                         start=(k == 0), stop=(k == KC - 1))
                    # PE gap-filler: second matmul of the previous row tile.
                    if prev is not None:
                        emit_mm2(prev)

                    m0 = statp.tile([128, 1], F32, tag="m0")
                    m1 = statp.tile([128, 1], F32, tag="m1")
                    nc.vector.reduce_max(m0[:], s0[:], axis=X)
                    nc.vector.reduce_max(m1[:], s1[:], axis=X)
                    negmax = statp.tile([128, 1], F32, tag="negmax")
                    nc.vector.tensor_max(negmax[:], m0[:], m1[:])
                    nc.vector.tensor_scalar_mul(negmax[:], negmax[:], -1.0)

                    is8 = i in FP8_TILES
                    pdt = FP8 if is8 else FP16
                    p = pp.tile([128, V], pdt, tag="p8" if is8 else "p")
                    es0 = statp.tile([128, 1], F32, tag="es0")
                    es1 = statp.tile([128, 1], F32, tag="es1")
                    nc.scalar.activation(p[:, 0:512], s0[:], Exp,
                                         bias=negmax[:], accum_out=es0[:])
                    nc.scalar.activation(p[:, 512:1024], s1[:], Exp,
                                         bias=negmax[:], accum_out=es1[:])
                    rec = statp.tile([128, 1], F32, tag="rec")
                    nc.vector.tensor_add(rec[:], es0[:], es1[:])
                    nc.vector.reciprocal(rec[:], rec[:])

                    if is8:
                        # fp8 PE transpose writes PSUM with element step 2
                        ptps = psPT.tile([128, JC, 256], FP8, tag="ptps8")
                        for j in range(JC):
                            nc.tensor.transpose(ptps[:, j, 0:256:2],
                                                p[:, ts(j, 128)], ident8[:])
                        ptt = ptp.tile([128, JC, 128], FP8, tag="pt8")
                        nc.vector.tensor_copy(ptt[:, 0:4, :],
                                              ptps[:, 0:4, 0:256:2])
                        nc.vector.tensor_copy(ptt[:, 4:8, :],
                                              ptps[:, 4:8, 0:256:2])
                    else:
                        ptps = psPT.tile([128, V], FP16, tag="ptps")
                        for j in range(JC):
                            nc.tensor.transpose(ptps[:, ts(j, 128)],
                                                p[:, ts(j, 128)], ident[:])
                        ptt = ptp.tile([128, V], FP16, tag="pt")
                        nc.vector.tensor_copy(ptt[:, 0:512], ptps[:, 0:512])
                        nc.vector.tensor_copy(ptt[:, 512:1024],
                                              ptps[:, 512:1024])
                    prev = (ptt, rec, i, is8)
                emit_mm2(prev)

            if loop_trips:
                with tc.For_i(0, loop_trips, 1):
                    for _ in range(8):
                        one_rep(0, reload_set=1)
                        one_rep(1, reload_set=0)
            else:
                for _ in range(reps):
                    one_rep(0)
    _split_multi_waits(nc)
    _dedup_ldweights(nc)
    return nc


def _shard_inputs(hidden_states, visual_hidden_state):
    H = np.ascontiguousarray(np.asarray(hidden_states, dtype=np.float32))
    Vh = np.ascontiguousarray(np.asarray(visual_hidden_state, dtype=np.float32))
    in_maps = []
    for b in range(B):
        Hb = H[b]                       # (L, D)
        Vb = Vh[b]                      # (V, D)
        ht = np.ascontiguousarray(
            Hb.reshape(NLT, LT, KC, 128).transpose(0, 3, 2, 1)
        ).reshape(NLT, 128, D).astype(np.float16)
        vt = np.ascontiguousarray(
            Vb.reshape(V, KC, 128).transpose(1, 2, 0)).astype(np.float16)
        import ml_dtypes
        vn = Vb.reshape(JC, 128, D).astype(np.float16)
        vnh = np.ascontiguousarray(
            Vb.reshape(JC, 128, D).transpose(1, 0, 2)
        ).astype(ml_dtypes.float8_e4m3)
        in_maps.append({"ht": ht, "vt": vt, "vn": vn, "vnh": vnh})
    return in_maps


def kernel(hidden_states, visual_hidden_state):
    in_maps = _shard_inputs(hidden_states, visual_hidden_state)
    nc = build_nc()
    res = run_bass_kernel_spmd(nc, in_maps, list(range(N_CORES)))
    return np.stack([
        res.results[c]["out"].reshape(L, D).astype(np.float32)
        for c in range(N_CORES)
    ])


if __name__ == "__main__":
    rng = np.random.default_rng(0)
    h = rng.standard_normal((B, L, D), dtype=np.float32)
    v = rng.standard_normal((B, V, D), dtype=np.float32)
    o = kernel(h, v)
    print("out", o.shape, o.dtype, o[0, 0, :4])


# revision 8
# speedup vs baseline: 1.0131x; 1.0131x over previous
"""BertFusion cross-attention kernel for 8x Trainium2 NeuronCores (final).

Problem (per batch element b):
    scores = H_b @ Vh_b^T          # (L, V) = (2048, 1024)
    probs  = softmax(scores, -1)
    out_b  = probs @ Vh_b          # (L, D) = (2048, 1024)

Sharding: data-parallel over batch (B=8 == n_cores), one batch element per
core. Host-side layouts (all fp16):
  - ht: H_b pre-transposed + tiled  [16, 128, 1024]  (mm1 stationary)
  - vt: Vh_b^T tiled                [8, 128, 1024]   (mm1 moving)
  - vn: Vh_b natural tiled          [8, 128, 1024]   (mm2 moving)

Device per core (flash-style over 16 row-tiles of 128 l-rows):
  mm1 (fp16, 1cyc/row): S[l,v] into 2 PSUM banks, double-buffered psS
  softmax along free v: DVE reduce_max -> ACT exp(bias=-max) -> Q fp16
  PE-transposes Q -> Q^T psum -> SBUF; mm2 (fp16): O = Q^T^T @ Vh
  ACT copy scaled by 1/sumexp -> fp16 out tile -> DMA.
PE order per iteration: mm1_i | mm2_{i-1} | transposes_i (gapless).

Timing loop: TWO full executions per For_i trip with double-buffered vt/vn
(sets A/B). Each rep's input DMA is issued at the start of the *other* rep,
so it overlaps compute and no rep starts on a cold SBUF; the For_i
drain+barrier is amortized over 2 executions. Each rep still pays its full
input DMA (ht per-tile + its whole vt/vn set) and writes the full output.
"""

import numpy as np

import concourse.bass as bass
import concourse.mybir as mybir
import concourse.tile as tile
from concourse.bass import ts
from concourse.bass_utils import run_bass_kernel_spmd
from concourse.masks import make_identity

# ---------------------------------------------------------------------------
# Workaround: the walrus build in this environment accepts only ONE sync-wait
# command per instruction, while Tile freely attaches several. Post-pass over
# the built module hoists extras onto standalone EventSemaphore carriers.
# ---------------------------------------------------------------------------
import bass_rust
from concourse.tile import ScopedClock


def _dist_drain_and_barrier(self, tick_clock, wait_clock):
    nc = self.nc
    drain_inst = nc.sync.drain()
    wait_clock.add_sem_waits(
        drain_inst.ins, ScopedClock({None: tick_clock.global_clock})
    )
    si = drain_inst.ins.sync_info
    if si is not None and si.on_wait and len(si.on_wait) > 1:
        waits = list(si.on_wait)
        si.on_wait = waits[:1]
        drain_inst.ins.sync_info = si
        engines = [
            mybir.EngineType.SP,
            mybir.EngineType.Activation,
            mybir.EngineType.DVE,
            mybir.EngineType.PE,
            mybir.EngineType.Pool,
        ]
        bb = nc.cur_bb.bb
        for n, w in enumerate(waits[1:]):
            c = mybir.InstEventSemaphore(name=f"I-esw-{nc.next_id()}")
            c.engine = engines[n % len(engines)]
            c.sync_info = bass_rust.SyncInfo(on_wait=[w], on_update=[])
            nc.register_instruction(c, overwrite=True)
            bb.add_instruction(c)

    nc.all_engine_barrier()
    assert self.sems is not None
    popped = nc._tile_sem_poison_stack.pop()
    assert popped is self._sem_poison
    nc.clear_and_free_semaphores(list(self.sems.allocated().values()))
    nc.all_engine_barrier()


tile.TileContext._drain_and_barrier = _dist_drain_and_barrier


def _split_multi_waits(nc, max_waits=1):
    for fn in nc.m.functions:
        for bb in fn.blocks:
            insts = bb.instructions
            need = any(
                i.sync_info is not None
                and i.sync_info.on_wait
                and len(i.sync_info.on_wait) > max_waits
                for i in insts
            )
            if not need:
                continue
            new = []
            for inst in insts:
                si = inst.sync_info
                if si is not None and si.on_wait and len(si.on_wait) > max_waits:
                    waits = list(si.on_wait)
                    extra, keep = waits[:-max_waits], waits[-max_waits:]
                    for w in extra:
                        c = mybir.InstEventSemaphore(name=f"I-esw-{nc.next_id()}")
                        c.engine = inst.engine
                        c.sync_info = bass_rust.SyncInfo(on_wait=[w], on_update=[])
                        new.append(c)
                    si.on_wait = keep
                    inst.sync_info = si
                new.append(inst)
            bb.instructions = new

def _dedup_ldweights(nc):
    """Drop InstLdweights that reload the stationary operand already resident
    in the PE array (the s0/s1 and o0/o1 half-pairs share their weights; the
    walrus ldw optimization that would elide these is disabled/broken in this
    build). Only syncless loads are dropped; any other PE instruction type
    invalidates the tracked weights."""
    for fn in nc.m.functions:
        for bb in fn.blocks:
            out = []
            last_w = None
            for inst in bb.instructions:
                tn = type(inst).__name__
                if inst.engine == mybir.EngineType.PE:
                    if tn == "InstLdweights":
                        si = inst.sync_info
                        has_sync = si is not None and (si.on_wait or si.on_update)
                        key = (str(inst.ins[0]),
                               getattr(inst, "is_transpose", None),
                               getattr(inst, "perf_mode", None))
                        if key == last_w and not has_sync:
                            continue
                        last_w = key
                    elif tn in ("InstMatmult", "InstEventSemaphore"):
                        pass
                    else:
                        last_w = None
                out.append(inst)
            bb.instructions = out


# ---------------------------------------------------------------------------

B, L, V, D = 8, 2048, 1024, 1024
LT = 128                # l-rows per tile
NLT = L // LT           # 16 row tiles
KC = D // 128           # 8 contraction chunks (mm1)
JC = V // 128           # 8 contraction chunks (mm2)
F32 = mybir.dt.float32
FP16 = mybir.dt.float16
FP8 = mybir.dt.float8e4
DR = mybir.MatmulPerfMode.DoubleRow
# Row tiles whose mm2 runs in single-fp8 DoubleRow (2x MACs/instr). 6/16
# tiles puts the global L2 error at ~1.65e-2 vs the 2e-2 gate (validated:
# the sqrt(f) scaling predicted 1.35e-2 at 4/16 and measured 1.349e-2).
FP8_TILES = frozenset((1, 3, 5, 7, 9, 12, 14))
N_CORES = 8


def build_nc(mm_dtype=mybir.dt.float16, reps=1, loop_trips=0,
             loop_reload=True):
    """Build the single-core Bass module (SPMD across 8 cores).

    loop_trips > 0: timing build — For_i loop, TWO executions per trip with
    A/B-alternating vt/vn buffers (cross-rep DMA prefetch). Divide the
    per-trip slope by 2 for per-execution time.
    """
    nc = bass.Bass("TRN2", target_bir_lowering=False, debug=False,
                   num_devices=N_CORES)
    mdt = mm_dtype
    ht = nc.dram_tensor("ht", [NLT, 128, D], mdt, kind="ExternalInput").ap()
    vt = nc.dram_tensor("vt", [KC, 128, V], mdt, kind="ExternalInput").ap()
    vn = nc.dram_tensor("vn", [JC, 128, D], FP16, kind="ExternalInput").ap()
    vnh = nc.dram_tensor("vnh", [128, JC, D], FP8, kind="ExternalInput").ap()
    out = nc.dram_tensor("out", [NLT, 128, D], FP16, kind="ExternalOutput").ap()

    Exp = mybir.ActivationFunctionType.Exp
    Copy = mybir.ActivationFunctionType.Copy
    X = mybir.AxisListType.X

    with tile.TileContext(nc) as tc:
        from contextlib import ExitStack
        with ExitStack() as st:
            cpool = st.enter_context(tc.tile_pool(name="const", bufs=1))
            vpool = st.enter_context(tc.tile_pool(name="vh", bufs=1))
            htp = st.enter_context(tc.tile_pool(name="htp", bufs=3))
            pp = st.enter_context(tc.tile_pool(name="pp", bufs=2))
            ptp = st.enter_context(tc.tile_pool(name="ptp", bufs=2))
            op = st.enter_context(tc.tile_pool(name="op", bufs=2))
            statp = st.enter_context(tc.tile_pool(name="statp", bufs=4))
            psS = st.enter_context(tc.tile_pool(name="psS", bufs=2, space="PSUM"))
            psPT = st.enter_context(tc.tile_pool(name="psPT", bufs=1, space="PSUM"))
            psO = st.enter_context(tc.tile_pool(name="psO", bufs=1, space="PSUM"))

            ident_f32 = cpool.tile([128, 128], F32, tag="ident_f32")
            make_identity(nc, ident_f32[:])
            ident = cpool.tile([128, 128], FP16, tag="ident")
            nc.vector.tensor_copy(ident[:], ident_f32[:])
            ident8 = cpool.tile([128, 128], FP8, tag="ident8")
            nc.vector.tensor_copy(ident8[:], ident_f32[:])

            two_sets = bool(loop_trips)
            sets = []
            for s in range(2 if two_sets else 1):
                vt_sb = [vpool.tile([128, V], mdt, tag=f"vt{s}_{k}",
                                    name=f"vt{s}_{k}") for k in range(KC)]
                vn_sb = [vpool.tile([128, D], FP16, tag=f"vn{s}_{j}",
                                    name=f"vn{s}_{j}") for j in range(JC)]
                vnh_sb = vpool.tile([128, JC, D], FP8, tag=f"vnh{s}",
                                    name=f"vnh{s}")
                sets.append((vt_sb, vn_sb, vnh_sb))

            def load_set(s, engine):
                vt_sb, vn_sb, vnh_sb = sets[s]
                for k in range(KC):
                    engine.dma_start(out=vt_sb[k][:], in_=vt[k])
                for j in range(JC):
                    engine.dma_start(out=vn_sb[j][:], in_=vn[j])
                engine.dma_start(out=vnh_sb[:], in_=vnh)

            # prologue: set 0 (and for the non-loop path, the only set)
            load_set(0, nc.sync)

            def load_ht(i):
                htt = htp.tile([128, D], mdt, tag="ht")
                nc.sync.dma_start(out=htt[:], in_=ht[i])
                return htt

            def one_rep(use_set, reload_set=None):
                vt_sb, vn_sb, vnh_sb = sets[use_set]
                prev = None
                ht_tiles = [load_ht(0), load_ht(1)]
                if reload_set is not None:
                    # prefetch the other set's inputs for the NEXT execution;
                    # issued on the ACT queue so it never delays ht streaming.
                    load_set(reload_set, nc.scalar)

                def emit_mm2(state):
                    ptt, rec, i, is8 = state
                    o0 = psO.tile([128, 512], F32, tag="o0")
                    o1 = psO.tile([128, 512], F32, tag="o1")
                    if is8:
                        for jp in range(JC // 2):
                            lhsT = ptt[:, 2 * jp:2 * jp + 2, :]
                            nc.tensor.matmul(o0[:], lhsT,
                                             vnh_sb[:, 2 * jp:2 * jp + 2, 0:512],
                                             start=(jp == 0),
                                             stop=(jp == JC // 2 - 1),
                                             perf_mode=DR)
                            nc.tensor.matmul(o1[:], lhsT,
                                             vnh_sb[:, 2 * jp:2 * jp + 2, 512:1024],
                                             start=(jp == 0),
                                             stop=(jp == JC // 2 - 1),
                                             perf_mode=DR)
                    else:
                        for j in range(JC):
                            lhsT = ptt[:, ts(j, 128)]
                            nc.tensor.matmul(o0[:], lhsT, vn_sb[j][:, 0:512],
                                             start=(j == 0), stop=(j == JC - 1))
                            nc.tensor.matmul(o1[:], lhsT, vn_sb[j][:, 512:1024],
                                             start=(j == 0), stop=(j == JC - 1))
                    ot = op.tile([128, D], FP16, tag="o")
                    nc.scalar.activation(ot[:, 0:512], o0[:], Copy, scale=rec[:])
                    nc.scalar.activation(ot[:, 512:1024], o1[:], Copy,
                                         scale=rec[:])
                    nc.sync.dma_start(out=out[i], in_=ot[:])

                for i in range(NLT):
                    htt = ht_tiles[i]
                    if i + 2 < NLT:
                        ht_tiles.append(load_ht(i + 2))
                    s0 = psS.tile([128, 512], F32, tag="s0")
                    s1 = psS.tile([128, 512], F32, tag="s1")
                    for k in range(KC):
                        lhsT = htt[:, ts(k, 128)]
                        nc.tensor.matmul(s0[:], lhsT,
                                         vt_sb[k][:, 0:512],
                                         start=(k == 0), stop=(k == KC - 1))
                        nc.tensor.matmul(s1[:], lhsT,
                                         vt_sb[k][:, 512:1024],
                                         start=(k == 0), stop=(k == KC - 1))
                    # PE gap-filler: second matmul of the previous row tile.
                    if prev is not None:
                        emit_mm2(prev)

                    m0 = statp.tile([128, 1], F32, tag="m0")
                    m1 = statp.tile([128, 1], F32, tag="m1")
                    nc.vector.reduce_max(m0[:], s0[:], axis=X)
                    nc.vector.reduce_max(m1[:], s1[:], axis=X)
                    negmax = statp.tile([128, 1], F32, tag="negmax")
                    nc.vector.tensor_max(negmax[:], m0[:], m1[:])
                    nc.vector.tensor_scalar_mul(negmax[:], negmax[:], -1.0)

                    is8 = i in FP8_TILES
                    pdt = FP8 if is8 else FP16
                    p0 = pp.tile([128, 512], pdt, tag="p80" if is8 else "p0",
                                 name="p0")
                    p1 = pp.tile([128, 512], pdt, tag="p81" if is8 else "p1",
                                 name="p1")
                    es0 = statp.tile([128, 1], F32, tag="es0")
                    es1 = statp.tile([128, 1], F32, tag="es1")
                    nc.scalar.activation(p0[:], s0[:], Exp,
                                         bias=negmax[:], accum_out=es0[:])
                    nc.scalar.activation(p1[:], s1[:], Exp,
                                         bias=negmax[:], accum_out=es1[:])
                    rec = statp.tile([128, 1], F32, tag="rec")
                    nc.vector.tensor_add(rec[:], es0[:], es1[:])
                    nc.vector.reciprocal(rec[:], rec[:])

                    if is8:
                        # fp8 PE transpose writes PSUM with element step 2
                        ptps = psPT.tile([128, JC, 256], FP8, tag="ptps8")
                        for j in range(JC):
                            psrc = p0 if j < 4 else p1
                            nc.tensor.transpose(ptps[:, j, 0:256:2],
                                                psrc[:, ts(j % 4, 128)],
                                                ident8[:])
                        ptt = ptp.tile([128, JC, 128], FP8, tag="pt8")
                        nc.vector.tensor_copy(ptt[:, 0:4, :],
                                              ptps[:, 0:4, 0:256:2])
                        nc.vector.tensor_copy(ptt[:, 4:8, :],
                                              ptps[:, 4:8, 0:256:2])
                    else:
                        ptps = psPT.tile([128, V], FP16, tag="ptps")
                        for j in range(JC):
                            psrc = p0 if j < 4 else p1
                            nc.tensor.transpose(ptps[:, ts(j, 128)],
                                                psrc[:, ts(j % 4, 128)],
                                                ident[:])
                        ptt = ptp.tile([128, V], FP16, tag="pt")
                        nc.vector.tensor_copy(ptt[:, 0:512], ptps[:, 0:512])
                        nc.vector.tensor_copy(ptt[:, 512:1024],
                                              ptps[:, 512:1024])
                    prev = (ptt, rec, i, is8)
                emit_mm2(prev)

            if loop_trips:
                with tc.For_i(0, loop_trips, 1):
                    for _ in range(8):
                        one_rep(0, reload_set=1)
                        one_rep(1, reload_set=0)
            else:
                for _ in range(reps):
                    one_rep(0)
    _split_multi_waits(nc)
    _dedup_ldweights(nc)
    return nc


def _shard_inputs(hidden_states, visual_hidden_state):
    H = np.ascontiguousarray(np.asarray(hidden_states, dtype=np.float32))
    Vh = np.ascontiguousarray(np.asarray(visual_hidden_state, dtype=np.float32))
    in_maps = []
    for b in range(B):
        Hb = H[b]                       # (L, D)
        Vb = Vh[b]                      # (V, D)
        ht = np.ascontiguousarray(
            Hb.reshape(NLT, LT, KC, 128).transpose(0, 3, 2, 1)
        ).reshape(NLT, 128, D).astype(np.float16)
        vt = np.ascontiguousarray(
            Vb.reshape(V, KC, 128).transpose(1, 2, 0)).astype(np.float16)
        import ml_dtypes
        vn = Vb.reshape(JC, 128, D).astype(np.float16)
        vnh = np.ascontiguousarray(
            Vb.reshape(JC, 128, D).transpose(1, 0, 2)
        ).astype(ml_dtypes.float8_e4m3)
        in_maps.append({"ht": ht, "vt": vt, "vn": vn, "vnh": vnh})
    return in_maps


def kernel(hidden_states, visual_hidden_state):
    in_maps = _shard_inputs(hidden_states, visual_hidden_state)
    nc = build_nc()
    res = run_bass_kernel_spmd(nc, in_maps, list(range(N_CORES)))
    return np.stack([
        res.results[c]["out"].reshape(L, D).astype(np.float32)
        for c in range(N_CORES)
    ])


if __name__ == "__main__":
    rng = np.random.default_rng(0)
    h = rng.standard_normal((B, L, D), dtype=np.float32)
    v = rng.standard_normal((B, V, D), dtype=np.float32)
    o = kernel(h, v)
    print("out", o.shape, o.dtype, o[0, 0, :4])
